# revision 14
# baseline (speedup 1.0000x reference)
"""AdaptiveGraphConv (Chebyshev K=3) Trainium2 kernel, 8-core data-parallel.

Math (per (batch,time) item, x_item [N,C]):
  M = D^-1/2 A D^-1/2  (normalized adjacency; L = I - M), M symmetric.
  T0 = x; T1 = Lx; T2 = 2L T1 - T0
  out = T0 W0 + T1 W1 + T2 W2 + b
      = x (W0+W1+W2) + (Mx)(-W1-4W2) + (M^2 x)(2W2) + b
M^2 is precomputed once (325x325), so both node-contractions read the same
node-major x and write channel-major results directly (no back-transposes):
  MX_cm[(b,c), i] = sum_j X_nm[j, (b,c)] * M[j, i]   (X_nm as stationary)
MX / M2X are stored t-major ([128, T, N], contiguous PSUM evictions); the
W-stage matmuls read them through an (n,t)-ordered strided AP, with the
output produced in n-blocks of 42 (504 = 42*12 columns per PSUM bank).
Sharding: data-parallel over batch dim B=64 -> 8 batches/core. Laplacian,
weights, bias replicated. No collectives.
"""
import os
import sys
import numpy as np

_TRN_REPO = "/opt/trn_rl_repo"
if _TRN_REPO not in sys.path:
    sys.path.insert(0, _TRN_REPO)


def _ensure_ntff_hook():
    """Make antenv.axon_hooks importable so NTFF profiling can register.

    The agent container's antenv stub lacks axon_hooks; trn_boot degrades
    silently without it. Writing the tiny registry module before concourse
    imports restores profiling. Harmless if already present.
    """
    src = (
        "_hook = None\n"
        "def set_axon_ntff_profile_hook(hook):\n"
        "    global _hook\n"
        "    _hook = hook\n"
        "def get_axon_ntff_profile_hook():\n"
        "    return _hook\n"
    )
    try:
        import antenv  # noqa
        base = os.path.dirname(antenv.__file__)
        path = os.path.join(base, "axon_hooks.py")
        if not os.path.exists(path):
            with open(path, "w") as f:
                f.write(src)
    except Exception:
        pass


_ensure_ntff_hook()

B, C, N, T, K = 64, 64, 325, 12, 3
NCORES = 8
B_LOC = B // NCORES          # 8 batches per core
NPAIRS = B_LOC // 2          # 4 pairs of batches
NT = N * T                   # 3900
CNT = [128, 128, 69]         # node chunk sizes (325 = 128+128+69)
NOFF = [0, 128, 256]
NBLK = 42                    # W-stage node-block (504 cols <= one PSUM bank)

_cache = {}


def _build():
    import concourse.bass as bass  # noqa
    import concourse.bacc as bacc
    import concourse.mybir as mybir
    import concourse.tile as tile
    from concourse import masks
    from contextlib import ExitStack

    f32 = mybir.dt.float32
    bf16 = mybir.dt.bfloat16
    ALU = mybir.AluOpType
    AF = mybir.ActivationFunctionType

    nc = bacc.Bacc("TRN2", target_bir_lowering=False, debug=False,
                   num_devices=NCORES)
    x_ext = nc.dram_tensor("x", [B_LOC, C, N, T], f32, kind="ExternalInput")
    adj_ext = nc.dram_tensor("adj", [N, N], f32, kind="ExternalInput")
    w_ext = nc.dram_tensor("W", [K, C, C], f32, kind="ExternalInput")
    b_ext = nc.dram_tensor("b", [C], f32, kind="ExternalInput")
    out_ext = nc.dram_tensor("out", [B_LOC, C, N, T], f32,
                             kind="ExternalOutput")

    with tile.TileContext(nc) as tc, ExitStack() as ctx:
        const = ctx.enter_context(tc.tile_pool(name="const", bufs=1))
        ps_t = ctx.enter_context(
            tc.tile_pool(name="ps_t", bufs=2, space="PSUM"))
        ps_m = ctx.enter_context(
            tc.tile_pool(name="ps_m", bufs=4, space="PSUM"))
        ps_w = ctx.enter_context(
            tc.tile_pool(name="ps_w", bufs=2, space="PSUM"))

        xs_pool = ctx.enter_context(tc.tile_pool(name="xs", bufs=2))
        nm_pool = ctx.enter_context(tc.tile_pool(name="nm", bufs=2))
        cm_pool = ctx.enter_context(tc.tile_pool(name="cm", bufs=2))
        out_pool = ctx.enter_context(tc.tile_pool(name="outp", bufs=2))

        state = {}

        def emit_loads(p):
            Xf = xs_pool.tile([128, N, T], f32, tag="xf", name="xf")
            if p == 0:
                for i in range(3):
                    nsl = slice(NOFF[i], NOFF[i] + CNT[i])
                    for h in (0, 1):
                        nc.sync.dma_start(Xf[64 * h: 64 * h + 64, nsl, :],
                                          x_ext.ap()[2 * p + h, :, nsl, :])
            else:
                for h in (0, 1):
                    nc.sync.dma_start(
                        Xf[64 * h: 64 * h + 64, :, :].rearrange(
                            "p n t -> p (n t)"),
                        x_ext.ap()[2 * p + h].rearrange("c n t -> c (n t)"))
            state[p] = Xf

        # adjacency first (it gates the whole M/M2 setup chain), then the
        # pair-0 input chunks, all on the sync HWDGE ring.
        Af = [const.tile([128, N], f32, tag=f"a{i}", name=f"a{i}")
              for i in range(3)]
        for i in range(3):
            nc.sync.dma_start(Af[i][: CNT[i], :],
                              adj_ext.ap()[NOFF[i]: NOFF[i] + CNT[i], :])
        emit_loads(0)

        idn = const.tile([128, 128], bf16)
        masks.make_identity(nc, idn[:])

        # ---- M = D^-1/2 A D^-1/2, three node-row tiles [cnt, 325] bf16 ----
        s_col = [const.tile([128, 1], f32, tag=f"s{i}", name=f"s{i}")
                 for i in range(3)]
        for i in range(3):
            d = const.tile([128, 1], f32, tag="dtmp")
            nc.vector.reduce_sum(d[: CNT[i], :], Af[i][: CNT[i], :],
                                 axis=mybir.AxisListType.X)
            nc.scalar.activation(d[: CNT[i], :], d[: CNT[i], :], AF.Sqrt)
            nc.vector.reciprocal(s_col[i][: CNT[i], :], d[: CNT[i], :])
        # s as a row vector [1, N] via tiny transposes (f32 path)
        idf = const.tile([128, 128], f32)
        masks.make_identity(nc, idf[:])
        ps_s = ps_m.tile([1, N], f32, tag="psm")
        for i in range(3):
            nc.tensor.matmul(ps_s[0:1, NOFF[i]: NOFF[i] + CNT[i]],
                             s_col[i][: CNT[i], 0:1], idf[: CNT[i], : CNT[i]],
                             is_transpose=True)
        s_row = const.tile([1, N], f32)
        nc.vector.tensor_copy(s_row[:], ps_s[:])
        # broadcast s_row to 128 partitions: ones[1,128].T @ s_row
        ones = const.tile([1, 128], f32)
        nc.vector.memset(ones[:], 1.0)
        ps_b = ps_m.tile([128, N], f32, tag="psm")
        nc.tensor.matmul(ps_b[:, :], ones[0:1, :], s_row[0:1, :])
        s_bc = const.tile([128, N], f32)
        nc.vector.tensor_copy(s_bc[:], ps_b[:])
        # M_i = (s_col * A * s_row) -> bf16
        M = [const.tile([128, N], bf16, tag=f"m{i}", name=f"m{i}")
             for i in range(3)]
        for i in range(3):
            nc.vector.tensor_mul(Af[i][: CNT[i], :], Af[i][: CNT[i], :],
                                 s_bc[: CNT[i], :])
            nc.vector.tensor_scalar_mul(M[i][: CNT[i], :], Af[i][: CNT[i], :],
                                        s_col[i][: CNT[i], 0:1])

        # ---- M2 = M @ M, three node-row tiles [cnt, 325] bf16 ----
        M2 = [const.tile([128, N], bf16, tag=f"m2_{i}", name=f"m2_{i}")
              for i in range(3)]
        for j in range(3):
            ps = ps_m.tile([128, N], f32, tag="psm")
            for k in range(3):
                nc.tensor.matmul(
                    ps[: CNT[j], :],
                    M[k][: CNT[k], NOFF[j]: NOFF[j] + CNT[j]],
                    M[k][: CNT[k], :],
                    start=(k == 0), stop=(k == 2))
            nc.scalar.activation(M2[j][: CNT[j], :], ps[: CNT[j], :], AF.Copy)

        # ---- weight combos as block-diagonal [128,128] bf16 (2 copies) ----
        # Wa = W0+W1+W2 ; Wb = -W1-4W2 ; Wc = 2W2
        Wsb = const.tile([128, K, C], f32)
        for h in (0, 1):
            nc.sync.dma_start(Wsb[64 * h: 64 * h + 64, :, :],
                              w_ext.ap().rearrange("k c d -> c k d"))
        Wa = const.tile([128, 128], bf16)
        Wb = const.tile([128, 128], bf16)
        Wc = const.tile([128, 128], bf16)
        for wt in (Wa, Wb, Wc):
            nc.gpsimd.memset(wt[:], 0.0)
        Wtmp = const.tile([128, C], f32)
        for h in (0, 1):
            r = slice(64 * h, 64 * h + 64)
            # Wa = (W0 + W1) + W2
            nc.vector.tensor_add(Wtmp[r, :], Wsb[r, 0, :], Wsb[r, 1, :])
            nc.vector.tensor_add(Wa[r, r], Wtmp[r, :], Wsb[r, 2, :])
            # Wb = (W2 * -4) - W1
            nc.vector.scalar_tensor_tensor(Wb[r, r], Wsb[r, 2, :], -4.0,
                                           Wsb[r, 1, :], ALU.mult,
                                           ALU.subtract)
            # Wc = 2*W2
            nc.vector.tensor_scalar_mul(Wc[r, r], Wsb[r, 2, :], 2.0)

        bias = const.tile([128, 1], f32)
        for h in (0, 1):
            nc.sync.dma_start(bias[64 * h: 64 * h + 64, :], b_ext.ap())

        def emit_convert(p):
            # f32 (n,t) -> bf16 (t,n): the reorder rides on the strided READ
            # (strided reads are cheap; strided writes are not).
            Xf = state.pop(p)
            Xs = xs_pool.tile([128, T, N], bf16, tag="xsb", name="xsb")
            for i in range(3):
                nsl = slice(NOFF[i], NOFF[i] + CNT[i])
                srcv = Xf[:, nsl, :].rearrange("p n t -> p t n")
                if i != 1:
                    nc.scalar.activation(Xs[:, :, nsl], srcv, AF.Copy)
                else:
                    nc.vector.tensor_copy(Xs[:, :, nsl], srcv)
            state[p] = Xs

        emit_convert(0)
        for p in range(NPAIRS):
            Xs = state.pop(p)

            # node-major X: 3 tiles [n<=128, T, 128=(2b,c)]
            XN = [nm_pool.tile([128, T, 128], bf16, tag=f"xn{i}",
                               name=f"xn{i}") for i in range(3)]
            for i in range(3):
                nsl = slice(NOFF[i], NOFF[i] + CNT[i])
                for tg in range(2):
                    ps = ps_t.tile([128, 6, 128], bf16, tag="pst")
                    for tt in range(6):
                        t = tg * 6 + tt
                        nc.tensor.matmul(
                            ps[: CNT[i], tt, :],
                            Xs[:, t, nsl],
                            idn[:], is_transpose=True)
                    if tg == 0:
                        nc.scalar.activation(
                            XN[i][: CNT[i], 0:6, :],
                            ps[: CNT[i], :, :], AF.Copy)
                    else:
                        nc.vector.tensor_copy(
                            XN[i][: CNT[i], 6:12, :],
                            ps[: CNT[i], :, :])

            # prefetch next pair
            if p + 1 < NPAIRS:
                emit_loads(p + 1)

            # M-apply: MX and M2X in channel-major, t-major storage.
            # psA[(2b,c), i] = sum_j XN[j][t,(2b,c)] * M[j][:, i]
            MX = cm_pool.tile([128, T, N], bf16, tag="mx")
            M2X = cm_pool.tile([128, T, N], bf16, tag="m2x")
            for t in range(T):
                psA = ps_m.tile([128, N], f32, tag="psm")
                psB = ps_m.tile([128, N], f32, tag="psm")
                for j in range(3):
                    lhsT = XN[j][: CNT[j], t, :]
                    nc.tensor.matmul(psA[:, :], lhsT, M[j][: CNT[j], :],
                                     start=(j == 0), stop=(j == 2))
                    nc.tensor.matmul(psB[:, :], lhsT, M2[j][: CNT[j], :],
                                     start=(j == 0), stop=(j == 2))
                if t % 2 == 0:
                    nc.vector.tensor_copy(MX[:, t, :], psA[:, :])
                    nc.scalar.activation(M2X[:, t, :], psB[:, :], AF.Copy)
                else:
                    nc.scalar.activation(MX[:, t, :], psA[:, :], AF.Copy)
                    nc.vector.tensor_copy(M2X[:, t, :], psB[:, :])

            if p + 1 < NPAIRS:
                emit_convert(p + 1)

            # W stage: out = Xs*Wa + MX*Wb + M2X*Wc + bias, in n-blocks.
            # Moving operands stream (t outer, n inner): all three rhs are
            # runs-of-nb contiguous reads; the psum holds (t, n) order and
            # the eviction does a strided PSUM read + contiguous SBUF write.
            # Output is split into two tiles so each half can DMA out as
            # soon as its four blocks are evicted.
            HALF = 4 * NBLK * T                      # 2016 cols (blocks 0-3)
            outA = out_pool.tile([128, HALF], f32, tag="outA")
            outB = out_pool.tile([128, NT - HALF], f32, tag="outB")
            for blk in range(8):
                nb0 = blk * NBLK
                nb = min(NBLK, N - nb0)
                ps = ps_w.tile([128, T, nb], f32, tag="psw")
                pw = ps[:, :, :]
                ra = Xs[:, :, nb0: nb0 + nb]
                rb = MX[:, :, nb0: nb0 + nb]
                rc = M2X[:, :, nb0: nb0 + nb]
                nc.tensor.matmul(pw, Wa[:], ra, start=True, stop=False)
                nc.tensor.matmul(pw, Wb[:], rb, start=False, stop=False)
                nc.tensor.matmul(pw, Wc[:], rc, start=False, stop=True)
                pr = pw.rearrange("p t n -> p n t")
                if blk < 4:
                    dst = outA[:, blk * NBLK * T: (blk + 1) * NBLK * T]
                else:
                    dst = outB[:, (blk - 4) * NBLK * T:
                               (blk - 4) * NBLK * T + nb * T]
                if blk % 2 == 0:
                    nc.scalar.activation(dst, pr, AF.Identity,
                                         bias=bias[:, 0:1])
                else:
                    nc.vector.tensor_scalar_add(dst, pr, bias[:, 0:1])

            out_hbm = [out_ext.ap()[2 * p + h].rearrange("c n t -> c (n t)")
                       for h in (0, 1)]
            for h in (0, 1):
                nc.sync.dma_start(out_hbm[h][:, :HALF],
                                  outA[64 * h: 64 * h + 64, :])
            for h in (0, 1):
                nc.scalar.dma_start(out_hbm[h][:, HALF:],
                                    outB[64 * h: 64 * h + 64, :])

    nc.compile()
    return nc


def _get_nc():
    if "nc" not in _cache:
        _cache["nc"] = _build()
    return _cache["nc"]


last_exec_time_ns = None
last_results = None


def kernel(x, adj, W, b):
    from concourse.bass_utils import run_bass_kernel_spmd

    global last_exec_time_ns, last_results
    nc = _get_nc()
    x = np.ascontiguousarray(x, dtype=np.float32)
    adj = np.ascontiguousarray(adj, dtype=np.float32)
    W = np.ascontiguousarray(W, dtype=np.float32)
    b = np.ascontiguousarray(b, dtype=np.float32)
    in_maps = [
        {"x": x[i * B_LOC: (i + 1) * B_LOC], "adj": adj, "W": W, "b": b}
        for i in range(NCORES)
    ]
    trace = bool(os.environ.get("KERNEL_TRACE"))
    res = run_bass_kernel_spmd(nc, in_maps, list(range(NCORES)), trace=trace)
    last_exec_time_ns = res.exec_time_ns
    last_results = res
    out = np.concatenate([res.results[i]["out"] for i in range(NCORES)],
                         axis=0)
    return out


# revision 15
# speedup vs baseline: 1.1340x; 1.1340x over previous
"""AdaptiveGraphConv (Chebyshev K=3) Trainium2 kernel, 8-core data-parallel.

Math (per (batch,time) item, x_item [N,C]):
  M = D^-1/2 A D^-1/2  (normalized adjacency; L = I - M), M symmetric.
  T0 = x; T1 = Lx; T2 = 2L T1 - T0
  out = T0 W0 + T1 W1 + T2 W2 + b
      = x (W0+W1+W2) + (Mx)(-W1-4W2) + (M^2 x)(2W2) + b
M^2 is precomputed once (325x325), so both node-contractions read the same
node-major x and write channel-major results directly (no back-transposes):
  MX_cm[(b,c), i] = sum_j X_nm[j, (b,c)] * M[j, i]   (X_nm as stationary)
MX / M2X are stored t-major ([128, T, N], contiguous PSUM evictions); the
W-stage matmuls read them through an (n,t)-ordered strided AP, with the
output produced in n-blocks of 42 (504 = 42*12 columns per PSUM bank).
Sharding: data-parallel over batch dim B=64 -> 8 batches/core. Laplacian,
weights, bias replicated. No collectives.
"""
import os
import sys
import numpy as np

_TRN_REPO = "/opt/trn_rl_repo"
if _TRN_REPO not in sys.path:
    sys.path.insert(0, _TRN_REPO)


def _ensure_ntff_hook():
    """Make antenv.axon_hooks importable so NTFF profiling can register.

    The agent container's antenv stub lacks axon_hooks; trn_boot degrades
    silently without it. Writing the tiny registry module before concourse
    imports restores profiling. Harmless if already present.
    """
    src = (
        "_hook = None\n"
        "def set_axon_ntff_profile_hook(hook):\n"
        "    global _hook\n"
        "    _hook = hook\n"
        "def get_axon_ntff_profile_hook():\n"
        "    return _hook\n"
    )
    try:
        import antenv  # noqa
        base = os.path.dirname(antenv.__file__)
        path = os.path.join(base, "axon_hooks.py")
        if not os.path.exists(path):
            with open(path, "w") as f:
                f.write(src)
    except Exception:
        pass


_ensure_ntff_hook()

B, C, N, T, K = 64, 64, 325, 12, 3
NCORES = 8
B_LOC = B // NCORES          # 8 batches per core
NPAIRS = B_LOC // 2          # 4 pairs of batches
NT = N * T                   # 3900
CNT = [128, 128, 69]         # node chunk sizes (325 = 128+128+69)
NOFF = [0, 128, 256]
NBLK = 42                    # W-stage node-block (504 cols <= one PSUM bank)

_cache = {}


def _build():
    import concourse.bass as bass  # noqa
    import concourse.bacc as bacc
    import concourse.mybir as mybir
    import concourse.tile as tile
    from concourse import masks
    from contextlib import ExitStack

    f32 = mybir.dt.float32
    bf16 = mybir.dt.bfloat16
    ALU = mybir.AluOpType
    AF = mybir.ActivationFunctionType

    nc = bacc.Bacc("TRN2", target_bir_lowering=False, debug=False,
                   num_devices=NCORES)
    x_ext = nc.dram_tensor("x", [B_LOC, C, N, T], f32, kind="ExternalInput")
    adj_ext = nc.dram_tensor("adj", [N, N], f32, kind="ExternalInput")
    w_ext = nc.dram_tensor("W", [K, C, C], f32, kind="ExternalInput")
    b_ext = nc.dram_tensor("b", [C], f32, kind="ExternalInput")
    out_ext = nc.dram_tensor("out", [B_LOC, C, N, T], f32,
                             kind="ExternalOutput")

    with tile.TileContext(nc) as tc, ExitStack() as ctx:
        const = ctx.enter_context(tc.tile_pool(name="const", bufs=1))
        ps_t = ctx.enter_context(
            tc.tile_pool(name="ps_t", bufs=2, space="PSUM"))
        ps_m = ctx.enter_context(
            tc.tile_pool(name="ps_m", bufs=4, space="PSUM"))
        ps_w = ctx.enter_context(
            tc.tile_pool(name="ps_w", bufs=2, space="PSUM"))

        xs_pool = ctx.enter_context(tc.tile_pool(name="xs", bufs=2))
        nm_pool = ctx.enter_context(tc.tile_pool(name="nm", bufs=2))
        cm_pool = ctx.enter_context(tc.tile_pool(name="cm", bufs=2))
        out_pool = ctx.enter_context(tc.tile_pool(name="outp", bufs=2))

        state = {}

        def emit_loads(p):
            Xf = xs_pool.tile([128, N, T], f32, tag="xf", name="xf")
            if p == 0:
                for i in range(3):
                    nsl = slice(NOFF[i], NOFF[i] + CNT[i])
                    for h in (0, 1):
                        nc.sync.dma_start(Xf[64 * h: 64 * h + 64, nsl, :],
                                          x_ext.ap()[2 * p + h, :, nsl, :])
            else:
                for h in (0, 1):
                    nc.sync.dma_start(
                        Xf[64 * h: 64 * h + 64, :, :].rearrange(
                            "p n t -> p (n t)"),
                        x_ext.ap()[2 * p + h].rearrange("c n t -> c (n t)"))
            state[p] = Xf

        # adjacency first (it gates the whole M/M2 setup chain), then the
        # pair-0 input chunks, all on the sync HWDGE ring.
        Af = [const.tile([128, N], f32, tag=f"a{i}", name=f"a{i}")
              for i in range(3)]
        for i in range(3):
            nc.sync.dma_start(Af[i][: CNT[i], :],
                              adj_ext.ap()[NOFF[i]: NOFF[i] + CNT[i], :])
        emit_loads(0)

        idn = const.tile([128, 128], bf16)
        masks.make_identity(nc, idn[:])

        # ---- M = D^-1/2 A D^-1/2, three node-row tiles [cnt, 325] bf16 ----
        s_col = [const.tile([128, 1], f32, tag=f"s{i}", name=f"s{i}")
                 for i in range(3)]
        for i in range(3):
            d = const.tile([128, 1], f32, tag="dtmp")
            nc.vector.reduce_sum(d[: CNT[i], :], Af[i][: CNT[i], :],
                                 axis=mybir.AxisListType.X)
            nc.scalar.activation(d[: CNT[i], :], d[: CNT[i], :], AF.Sqrt)
            nc.vector.reciprocal(s_col[i][: CNT[i], :], d[: CNT[i], :])
        # s as a row vector [1, N] via tiny transposes (f32 path)
        idf = const.tile([128, 128], f32)
        masks.make_identity(nc, idf[:])
        ps_s = ps_m.tile([1, N], f32, tag="psm")
        for i in range(3):
            nc.tensor.matmul(ps_s[0:1, NOFF[i]: NOFF[i] + CNT[i]],
                             s_col[i][: CNT[i], 0:1], idf[: CNT[i], : CNT[i]],
                             is_transpose=True)
        s_row = const.tile([1, N], f32)
        nc.vector.tensor_copy(s_row[:], ps_s[:])
        # broadcast s_row to 128 partitions: ones[1,128].T @ s_row
        ones = const.tile([1, 128], f32)
        nc.vector.memset(ones[:], 1.0)
        ps_b = ps_m.tile([128, N], f32, tag="psm")
        nc.tensor.matmul(ps_b[:, :], ones[0:1, :], s_row[0:1, :])
        s_bc = const.tile([128, N], f32)
        nc.vector.tensor_copy(s_bc[:], ps_b[:])
        # M_i = (s_col * A * s_row) -> bf16
        M = [const.tile([128, N], bf16, tag=f"m{i}", name=f"m{i}")
             for i in range(3)]
        for i in range(3):
            nc.vector.tensor_mul(Af[i][: CNT[i], :], Af[i][: CNT[i], :],
                                 s_bc[: CNT[i], :])
            nc.vector.tensor_scalar_mul(M[i][: CNT[i], :], Af[i][: CNT[i], :],
                                        s_col[i][: CNT[i], 0:1])

        # ---- M2 = M @ M, three node-row tiles [cnt, 325] bf16 ----
        M2 = [const.tile([128, N], bf16, tag=f"m2_{i}", name=f"m2_{i}")
              for i in range(3)]
        for j in range(3):
            ps = ps_m.tile([128, N], f32, tag="psm")
            for k in range(3):
                nc.tensor.matmul(
                    ps[: CNT[j], :],
                    M[k][: CNT[k], NOFF[j]: NOFF[j] + CNT[j]],
                    M[k][: CNT[k], :],
                    start=(k == 0), stop=(k == 2))
            nc.scalar.activation(M2[j][: CNT[j], :], ps[: CNT[j], :], AF.Copy)

        # ---- weight combos as block-diagonal [128,128] bf16 (2 copies) ----
        # Wa = W0+W1+W2 ; Wb = -W1-4W2 ; Wc = 2W2
        Wsb = const.tile([128, K, C], f32)
        for h in (0, 1):
            nc.sync.dma_start(Wsb[64 * h: 64 * h + 64, :, :],
                              w_ext.ap().rearrange("k c d -> c k d"))
        Wa = const.tile([128, 128], bf16)
        Wb = const.tile([128, 128], bf16)
        Wc = const.tile([128, 128], bf16)
        for wt in (Wa, Wb, Wc):
            nc.gpsimd.memset(wt[:], 0.0)
        Wtmp = const.tile([128, C], f32)
        for h in (0, 1):
            r = slice(64 * h, 64 * h + 64)
            # Wa = (W0 + W1) + W2
            nc.vector.tensor_add(Wtmp[r, :], Wsb[r, 0, :], Wsb[r, 1, :])
            nc.vector.tensor_add(Wa[r, r], Wtmp[r, :], Wsb[r, 2, :])
            # Wb = (W2 * -4) - W1
            nc.vector.scalar_tensor_tensor(Wb[r, r], Wsb[r, 2, :], -4.0,
                                           Wsb[r, 1, :], ALU.mult,
                                           ALU.subtract)
            # Wc = 2*W2
            nc.vector.tensor_scalar_mul(Wc[r, r], Wsb[r, 2, :], 2.0)

        bias = const.tile([128, 1], f32)
        for h in (0, 1):
            nc.sync.dma_start(bias[64 * h: 64 * h + 64, :], b_ext.ap())

        def emit_convert(p):
            # f32 (n,t) -> bf16 (t,n): the reorder rides on the strided READ
            # (strided reads are cheap; strided writes are not).
            Xf = state.pop(p)
            Xs = xs_pool.tile([128, T, N], bf16, tag="xsb", name="xsb")
            for i in range(3):
                nsl = slice(NOFF[i], NOFF[i] + CNT[i])
                srcv = Xf[:, nsl, :].rearrange("p n t -> p t n")
                if i != 1:
                    nc.scalar.activation(Xs[:, :, nsl], srcv, AF.Copy)
                else:
                    nc.vector.tensor_copy(Xs[:, :, nsl], srcv)
            state[p] = Xs

        emit_convert(0)
        for p in range(NPAIRS):
            Xs = state.pop(p)

            # node-major X: 3 tiles [n<=128, T, 128=(2b,c)]
            XN = [nm_pool.tile([128, T, 128], bf16, tag=f"xn{i}",
                               name=f"xn{i}") for i in range(3)]
            for i in range(3):
                nsl = slice(NOFF[i], NOFF[i] + CNT[i])
                for tg in range(2):
                    ps = ps_t.tile([128, 6, 128], bf16, tag="pst")
                    for tt in range(6):
                        t = tg * 6 + tt
                        nc.tensor.matmul(
                            ps[: CNT[i], tt, :],
                            Xs[:, t, nsl],
                            idn[:], is_transpose=True)
                    if tg == 0:
                        nc.scalar.activation(
                            XN[i][: CNT[i], 0:6, :],
                            ps[: CNT[i], :, :], AF.Copy)
                    else:
                        nc.vector.tensor_copy(
                            XN[i][: CNT[i], 6:12, :],
                            ps[: CNT[i], :, :])

            # prefetch next pair
            if p + 1 < NPAIRS:
                emit_loads(p + 1)

            # M-apply: MX and M2X in channel-major, t-major storage.
            # psA[(2b,c), i] = sum_j XN[j][t,(2b,c)] * M[j][:, i]
            MX = cm_pool.tile([128, T, N], bf16, tag="mx")
            M2X = cm_pool.tile([128, T, N], bf16, tag="m2x")
            for t in range(T):
                psA = ps_m.tile([128, N], f32, tag="psm")
                psB = ps_m.tile([128, N], f32, tag="psm")
                for j in range(3):
                    lhsT = XN[j][: CNT[j], t, :]
                    nc.tensor.matmul(psA[:, :], lhsT, M[j][: CNT[j], :],
                                     start=(j == 0), stop=(j == 2))
                    nc.tensor.matmul(psB[:, :], lhsT, M2[j][: CNT[j], :],
                                     start=(j == 0), stop=(j == 2))
                if t % 2 == 0:
                    nc.vector.tensor_copy(MX[:, t, :], psA[:, :])
                    nc.scalar.activation(M2X[:, t, :], psB[:, :], AF.Copy)
                else:
                    nc.scalar.activation(MX[:, t, :], psA[:, :], AF.Copy)
                    nc.vector.tensor_copy(M2X[:, t, :], psB[:, :])

            if p + 1 < NPAIRS:
                emit_convert(p + 1)

            # W stage: out = Xs*Wa + MX*Wb + M2X*Wc + bias, in n-blocks.
            # Moving operands stream (t outer, n inner): all three rhs are
            # runs-of-nb contiguous reads; the psum holds (t, n) order and
            # the eviction does a strided PSUM read + contiguous SBUF write.
            # Output is split into two tiles so each half can DMA out as
            # soon as its four blocks are evicted.
            HALF = 4 * NBLK * T                      # 2016 cols (blocks 0-3)
            outA = out_pool.tile([128, HALF], f32, tag="outA")
            outB = out_pool.tile([128, NT - HALF], f32, tag="outB")
            for blk in range(8):
                nb0 = blk * NBLK
                nb = min(NBLK, N - nb0)
                ps = ps_w.tile([128, T, nb], f32, tag="psw")
                pw = ps[:, :, :]
                ra = Xs[:, :, nb0: nb0 + nb]
                rb = MX[:, :, nb0: nb0 + nb]
                rc = M2X[:, :, nb0: nb0 + nb]
                nc.tensor.matmul(pw, Wa[:], ra, start=True, stop=False)
                nc.tensor.matmul(pw, Wb[:], rb, start=False, stop=False)
                nc.tensor.matmul(pw, Wc[:], rc, start=False, stop=True)
                if blk < 4:
                    dst = outA[:, blk * NBLK * T: (blk + 1) * NBLK * T]
                else:
                    dst = outB[:, (blk - 4) * NBLK * T:
                               (blk - 4) * NBLK * T + nb * T]
                # evict split across both engines so the PSUM bank frees
                # in half the time (ps_w has only 2 bufs)
                nh = nb // 2
                pr0 = ps[:, :, :nh].rearrange("p t n -> p n t")
                pr1 = ps[:, :, nh:].rearrange("p t n -> p n t")
                nc.scalar.activation(dst[:, : nh * T], pr0, AF.Identity,
                                     bias=bias[:, 0:1])
                nc.vector.tensor_scalar_add(dst[:, nh * T:], pr1,
                                            bias[:, 0:1])

            out_hbm = [out_ext.ap()[2 * p + h].rearrange("c n t -> c (n t)")
                       for h in (0, 1)]
            for h in (0, 1):
                nc.sync.dma_start(out_hbm[h][:, :HALF],
                                  outA[64 * h: 64 * h + 64, :])
            for h in (0, 1):
                nc.scalar.dma_start(out_hbm[h][:, HALF:],
                                    outB[64 * h: 64 * h + 64, :])

    nc.compile()
    return nc


def _get_nc():
    if "nc" not in _cache:
        _cache["nc"] = _build()
    return _cache["nc"]


last_exec_time_ns = None
last_results = None


def kernel(x, adj, W, b):
    from concourse.bass_utils import run_bass_kernel_spmd

    global last_exec_time_ns, last_results
    nc = _get_nc()
    x = np.ascontiguousarray(x, dtype=np.float32)
    adj = np.ascontiguousarray(adj, dtype=np.float32)
    W = np.ascontiguousarray(W, dtype=np.float32)
    b = np.ascontiguousarray(b, dtype=np.float32)
    in_maps = [
        {"x": x[i * B_LOC: (i + 1) * B_LOC], "adj": adj, "W": W, "b": b}
        for i in range(NCORES)
    ]
    trace = bool(os.environ.get("KERNEL_TRACE"))
    res = run_bass_kernel_spmd(nc, in_maps, list(range(NCORES)), trace=trace)
    last_exec_time_ns = res.exec_time_ns
    last_results = res
    out = np.concatenate([res.results[i]["out"] for i in range(NCORES)],
                         axis=0)
    return out
